# revision 32
# baseline (speedup 1.0000x reference)
"""Trainium2 Bass kernel for ConcatHandshaking.

out[b, p, :] = tanh(hidden[b, i_p] @ W1.T + hidden[b, j_p] @ W2.T + fc_b)
for the S*(S+1)/2 upper-triangular pairs (i_p, j_p), i-major order.

Device layout: output features (H=768) on SBUF partitions, pair index on the
free dim.  Then the pair-add is `q2T[:, j] + p1T[:, i]` where the second term
is a per-partition scalar -> one DVE tensor_scalar_add per triu segment,
fused bias, one big ACT tanh per output chunk, large contiguous DMA writes.

Sharding (8 cores): core k handles batch b = k//2 and output-feature rows
[384*(k%2), 384*(k%2)+384) -> 3 stripes of [128 features, 32896 pairs] each.
Per-core DRAM output is (3, 128, 32896); host reassembles + transposes.

Matmul operands ship as one bf16 tensor (PE 4x faster than f32; rel err
~1e-3 after f32 PSUM accumulation); fcb/zeros ship in a tiny f32 tensor.
The first stripe uses small leading chunks so the first output DMA starts
~12us in instead of waiting on a full 8224-wide chunk.
"""

import sys

import numpy as np

for _p in ("/opt/trn_rl_repo",):
    if _p not in sys.path:
        sys.path.insert(0, _p)

B, S, H = 4, 256, 768
P = S * (S + 1) // 2  # 32896
KT = H // 128  # 6 k-tiles
OC = 3  # o-chunks (of 128) per core
# bf16 packed matmul input columns: [ ht (S) | w1t (384) | w2t (384) ]
W1C = S
W2C = S + 128 * OC
IC16 = S + 2 * 128 * OC  # 1024
BIGCHUNK = 8224
SMALL = 2056
# segments with i < FUSE_T run as single ACT ops (tanh with per-partition
# bias = p1[:, i]) writing ot2 directly -- no DVE pass, no extra SBUF hops.
# Short segments (i >= FUSE_T) would drown in ACT instruction overhead, so
# they keep the add + one-big-tanh path, with the adds split between DVE
# and the otherwise-idle GPSIMD engine (band GPS_LO <= i < GPS_HI).
FUSE_T = 32
GPS_LO = 128
GPS_HI = 208

_NC_CACHE = {}
LAST = {}


def _stripe_chunks(c):
    if c == 0:
        # geometric-ish leading chunks: first output DMA launches early and
        # the stream never stalls waiting on one big chunk's DVE+ACT latency
        return [1028, 1028, 2056, 2056, 2056] + [4112] * 6
    return [BIGCHUNK] * 4


def _chunk_pieces(chunk_list):
    """Split triu segments along chunk boundaries.

    Returns per-chunk lists of (i, src0, src1, dst0):
    chunk[:, dst0:dst0+(src1-src0)] = q2T[:, src0:src1] + p1T[:, i].
    """
    bounds = [0]
    for sz in chunk_list:
        bounds.append(bounds[-1] + sz)
    assert bounds[-1] == P
    pieces = [[] for _ in chunk_list]
    off = 0
    for i in range(S):
        seg0, seg1 = off, off + (S - i)
        off = seg1
        for ci, (c0, c1) in enumerate(zip(bounds[:-1], bounds[1:])):
            s = max(seg0, c0)
            e = min(seg1, c1)
            if e > s:
                src0 = i + (s - seg0)  # free index in q2T is j itself
                pieces[ci].append((i, src0, src0 + (e - s), s - c0))
    return pieces


def _build_nc(loop_k=None, fuse_t=None, gps_lo=None, gps_hi=None):
    if fuse_t is None:
        fuse_t = FUSE_T
    if gps_lo is None:
        gps_lo = GPS_LO
    if gps_hi is None:
        gps_hi = GPS_HI
    import contextlib

    import concourse.bacc as bacc
    import concourse.mybir as mybir
    import concourse.tile as tile

    f32 = mybir.dt.float32
    bf16 = mybir.dt.bfloat16
    # Bacc (not raw Bass): its compile() runs generate_event_semaphores,
    # which splits multi-sem waits to satisfy TRN2's 1-wait-per-instruction.
    nc = bacc.Bacc()

    inp16_d = nc.declare_dram_parameter("inp16", [H, IC16], bf16, isOutput=False)
    # f32 side data: col 0 = fcb (rows 0:384), col 1 = zeros
    aux_d = nc.declare_dram_parameter("aux", [H, 2], f32, isOutput=False)
    out_d = nc.declare_dram_parameter("out", [OC, 128, P], f32, isOutput=True)

    Tanh = mybir.ActivationFunctionType.Tanh

    with tile.TileContext(nc) as tc:
        with (
            tc.tile_pool(name="const", bufs=1) as cpool,
            tc.tile_pool(name="mm", bufs=2, space="PSUM") as mpool,
            tc.tile_pool(name="outp", bufs=2) as opool,
            tc.tile_pool(name="outp2", bufs=3) as opool2,
            tc.For_i(0, loop_k, 1) if loop_k else contextlib.nullcontext(),
        ):
            # one DMA per k-tile so matmul kk can start as soon as its
            # k-tile lands (pipelines the load under the matmul chain)
            inp_b = cpool.tile([128, KT * IC16], bf16, name="inp_b")
            for kk in range(KT):
                nc.sync.dma_start(
                    inp_b[:, kk * IC16 : (kk + 1) * IC16],
                    inp16_d[kk * 128 : (kk + 1) * 128, :],
                )
            aux_b = cpool.tile([128, KT * 2], f32, name="aux_b")
            nc.sync.dma_start(
                aux_b[:].rearrange("p (t c) -> p t c", t=KT),
                aux_d.rearrange("(t p) c -> p t c", p=128),
            )
            # block kk occupies cols [kk*IC16, (kk+1)*IC16)
            ht_t = [inp_b[:, kk * IC16 : kk * IC16 + S] for kk in range(KT)]
            fcb_t = [aux_b[:, c * 2 : c * 2 + 1] for c in range(OC)]

            for c in range(OC):
                pm1 = mpool.tile([128, S], f32, name="pm1")
                pm2 = mpool.tile([128, S], f32, name="pm2")
                for kk in range(KT):
                    nc.tensor.matmul(
                        pm1[:],
                        inp_b[
                            :, kk * IC16 + W1C + c * 128 : kk * IC16 + W1C + (c + 1) * 128
                        ],
                        ht_t[kk],
                        start=(kk == 0),
                        stop=(kk == KT - 1),
                    )
                for kk in range(KT):
                    nc.tensor.matmul(
                        pm2[:],
                        inp_b[
                            :, kk * IC16 + W2C + c * 128 : kk * IC16 + W2C + (c + 1) * 128
                        ],
                        ht_t[kk],
                        start=(kk == 0),
                        stop=(kk == KT - 1),
                    )
                p1 = cpool.tile([128, S], f32, name=f"p1_{c}")
                q2 = cpool.tile([128, S], f32, name=f"q2_{c}")
                nc.vector.tensor_copy(p1[:], pm1[:])
                nc.vector.tensor_scalar_add(q2[:], pm2[:], fcb_t[c])

                chunk_list = _stripe_chunks(c)
                pieces = _chunk_pieces(chunk_list)
                coff = 0
                for ci, csz in enumerate(chunk_list):
                    fused = [p for p in pieces[ci] if p[0] < fuse_t]
                    rest = [p for p in pieces[ci] if p[0] >= fuse_t]
                    ot2 = opool2.tile([128, BIGCHUNK], f32, name="ot2")
                    if rest:
                        # adds for the short segments (DVE or GPSIMD by
                        # band), then one tanh over their contiguous extent
                        ot = opool.tile([128, BIGCHUNK], f32, name="ot")
                        for (i, s0, s1, d0) in rest:
                            eng = (
                                nc.gpsimd
                                if gps_lo <= i < gps_hi
                                else nc.vector
                            )
                            eng.tensor_scalar_add(
                                ot[:, d0 : d0 + (s1 - s0)],
                                q2[:, s0:s1],
                                p1[:, i : i + 1],
                            )
                        r0 = rest[0][3]
                        r1 = rest[-1][3] + (rest[-1][2] - rest[-1][1])
                        nc.scalar.activation(ot2[:, r0:r1], ot[:, r0:r1], Tanh)
                    for (i, s0, s1, d0) in fused:
                        nc.scalar.activation(
                            ot2[:, d0 : d0 + (s1 - s0)],
                            q2[:, s0:s1],
                            Tanh,
                            bias=p1[:, i : i + 1],
                        )
                    nc.sync.dma_start(
                        out_d[c, :, coff : coff + csz], ot2[:, :csz]
                    )
                    coff += csz
    nc.compile()
    return nc


def _get_nc():
    if "nc" not in _NC_CACHE:
        _NC_CACHE["nc"] = _build_nc()
    return _NC_CACHE["nc"]


def _make_in_maps(hidden_state, fc_w, fc_b):
    import ml_dtypes

    in_maps = []
    for k in range(8):
        b, h0 = k // 2, 384 * (k % 2)
        inp16 = np.empty((H, IC16), dtype=ml_dtypes.bfloat16)
        inp16[:, :S] = hidden_state[b].T.astype(ml_dtypes.bfloat16)
        inp16[:, W1C : W1C + 384] = fc_w[h0 : h0 + 384, :H].T.astype(
            ml_dtypes.bfloat16
        )
        inp16[:, W2C : W2C + 384] = fc_w[h0 : h0 + 384, H:].T.astype(
            ml_dtypes.bfloat16
        )
        aux = np.zeros((H, 2), dtype=np.float32)
        aux[: 128 * OC, 0] = fc_b[h0 : h0 + 384]
        in_maps.append(dict(inp16=inp16, aux=aux))
    return in_maps


def kernel(hidden_state, fc_w, fc_b, _trace=False, **_trace_kwargs):
    from concourse.bass_utils import run_bass_kernel_spmd

    hidden_state = np.asarray(hidden_state, dtype=np.float32)
    fc_w = np.asarray(fc_w, dtype=np.float32)
    fc_b = np.asarray(fc_b, dtype=np.float32)

    in_maps = _make_in_maps(hidden_state, fc_w, fc_b)
    nc = _get_nc()
    res = run_bass_kernel_spmd(
        nc, in_maps, core_ids=list(range(8)), trace=_trace, **_trace_kwargs
    )
    LAST["res"] = res

    full = np.empty((B, H, P), dtype=np.float32)
    for k in range(8):
        b, h0 = k // 2, 384 * (k % 2)
        full[b, h0 : h0 + 384] = res.results[k]["out"].reshape(384, P)
    return np.ascontiguousarray(full.transpose(0, 2, 1))


# revision 36
# speedup vs baseline: 2.4742x; 2.4742x over previous
"""Trainium2 Bass kernel for ConcatHandshaking.

out[b, p, :] = tanh(hidden[b, i_p] @ W1.T + hidden[b, j_p] @ W2.T + fc_b)
for the S*(S+1)/2 upper-triangular pairs (i_p, j_p), i-major order.

Device layout: output features (H=768) on SBUF partitions, pair index on the
free dim.  Then the pair-add is `q2T[:, j] + p1T[:, i]` where the second term
is a per-partition scalar -> one DVE tensor_scalar_add per triu segment,
fused bias, one big ACT tanh per output chunk, large contiguous DMA writes.

Sharding (8 cores): core k handles batch b = k//2 and output-feature rows
[384*(k%2), 384*(k%2)+384) -> 3 stripes of [128 features, 32896 pairs] each.
Per-core DRAM output is (3, 128, 32896); host reassembles + transposes.

Matmul operands ship as one bf16 tensor (PE 4x faster than f32; rel err
~1e-3 after f32 PSUM accumulation); fcb/zeros ship in a tiny f32 tensor.
The first stripe uses small leading chunks so the first output DMA starts
~12us in instead of waiting on a full 8224-wide chunk.
"""

import sys

import numpy as np

for _p in ("/opt/trn_rl_repo",):
    if _p not in sys.path:
        sys.path.insert(0, _p)

B, S, H = 4, 256, 768
P = S * (S + 1) // 2  # 32896
KT = H // 128  # 6 k-tiles
OC = 3  # o-chunks (of 128) per core
# bf16 packed matmul input columns: [ ht (S) | w1t (384) | w2t (384) ]
W1C = S
W2C = S + 128 * OC
IC16 = S + 2 * 128 * OC  # 1024
BIGCHUNK = 8224
SMALL = 2056
# segments with i < FUSE_T run as single ACT ops (tanh with per-partition
# bias = p1[:, i]) writing ot2 directly -- no DVE pass, no extra SBUF hops.
# Short segments (i >= FUSE_T) would drown in ACT instruction overhead, so
# they keep the add + one-big-tanh path on DVE.  (A GPSIMD band was tried
# and is ~6x slower per op on real HW than the cost model claims -- unused.)
# Consecutive full segments are merged in PAIRS into one DVE tensor_tensor
# with an overlapping-window AP: row g reads q2[i+g : i+g+L], adds
# p1[:, i+g] (free-step-0 broadcast).  Row 1 writes one spill element that
# the next instruction's first write repairs (same-engine program order).
FUSE_T = 32
GPS_LO = 128
GPS_HI = 128

_NC_CACHE = {}
LAST = {}


def _stripe_chunks(c):
    if c == 0:
        # geometric-ish leading chunks: first output DMA launches early and
        # the stream never stalls waiting on one big chunk's DVE+ACT latency
        return [1028, 1028, 2056, 2056, 2056] + [4112] * 6
    return [BIGCHUNK] * 4


def _chunk_pieces(chunk_list):
    """Split triu segments along chunk boundaries.

    Returns per-chunk lists of (i, src0, src1, dst0):
    chunk[:, dst0:dst0+(src1-src0)] = q2T[:, src0:src1] + p1T[:, i].
    """
    bounds = [0]
    for sz in chunk_list:
        bounds.append(bounds[-1] + sz)
    assert bounds[-1] == P
    pieces = [[] for _ in chunk_list]
    off = 0
    for i in range(S):
        seg0, seg1 = off, off + (S - i)
        off = seg1
        for ci, (c0, c1) in enumerate(zip(bounds[:-1], bounds[1:])):
            s = max(seg0, c0)
            e = min(seg1, c1)
            if e > s:
                src0 = i + (s - seg0)  # free index in q2T is j itself
                pieces[ci].append((i, src0, src0 + (e - s), s - c0))
    return pieces


def _build_nc(loop_k=None, fuse_t=None, gps_lo=None, gps_hi=None):
    if fuse_t is None:
        fuse_t = FUSE_T
    if gps_lo is None:
        gps_lo = GPS_LO
    if gps_hi is None:
        gps_hi = GPS_HI
    import contextlib

    import concourse.bacc as bacc
    import concourse.bass as bass
    import concourse.mybir as mybir
    import concourse.tile as tile

    def _sub_ap(t, off, dims):
        return bass.AP(tensor=t.tensor, offset=t.offset + off, ap=[t.ap[0]] + dims)

    f32 = mybir.dt.float32
    bf16 = mybir.dt.bfloat16
    # Bacc (not raw Bass): its compile() runs generate_event_semaphores,
    # which splits multi-sem waits to satisfy TRN2's 1-wait-per-instruction.
    nc = bacc.Bacc()

    inp16_d = nc.declare_dram_parameter("inp16", [H, IC16], bf16, isOutput=False)
    # f32 side data: col 0 = fcb (rows 0:384), col 1 = zeros
    aux_d = nc.declare_dram_parameter("aux", [H, 2], f32, isOutput=False)
    out_d = nc.declare_dram_parameter("out", [OC, 128, P], f32, isOutput=True)

    Tanh = mybir.ActivationFunctionType.Tanh

    with tile.TileContext(nc) as tc:
        with (
            tc.tile_pool(name="const", bufs=1) as cpool,
            tc.tile_pool(name="mm", bufs=2, space="PSUM") as mpool,
            tc.tile_pool(name="outp", bufs=2) as opool,
            tc.tile_pool(name="outp2", bufs=3) as opool2,
            tc.For_i(0, loop_k, 1) if loop_k else contextlib.nullcontext(),
        ):
            # one DMA per k-tile so matmul kk can start as soon as its
            # k-tile lands (pipelines the load under the matmul chain)
            inp_b = cpool.tile([128, KT * IC16], bf16, name="inp_b")
            for kk in range(KT):
                nc.sync.dma_start(
                    inp_b[:, kk * IC16 : (kk + 1) * IC16],
                    inp16_d[kk * 128 : (kk + 1) * 128, :],
                )
            aux_b = cpool.tile([128, KT * 2], f32, name="aux_b")
            nc.sync.dma_start(
                aux_b[:].rearrange("p (t c) -> p t c", t=KT),
                aux_d.rearrange("(t p) c -> p t c", p=128),
            )
            # block kk occupies cols [kk*IC16, (kk+1)*IC16)
            ht_t = [inp_b[:, kk * IC16 : kk * IC16 + S] for kk in range(KT)]
            fcb_t = [aux_b[:, c * 2 : c * 2 + 1] for c in range(OC)]

            for c in range(OC):
                pm1 = mpool.tile([128, S], f32, name="pm1")
                pm2 = mpool.tile([128, S], f32, name="pm2")
                for kk in range(KT):
                    nc.tensor.matmul(
                        pm1[:],
                        inp_b[
                            :, kk * IC16 + W1C + c * 128 : kk * IC16 + W1C + (c + 1) * 128
                        ],
                        ht_t[kk],
                        start=(kk == 0),
                        stop=(kk == KT - 1),
                    )
                for kk in range(KT):
                    nc.tensor.matmul(
                        pm2[:],
                        inp_b[
                            :, kk * IC16 + W2C + c * 128 : kk * IC16 + W2C + (c + 1) * 128
                        ],
                        ht_t[kk],
                        start=(kk == 0),
                        stop=(kk == KT - 1),
                    )
                p1 = cpool.tile([128, S], f32, name=f"p1_{c}")
                # one pad column: paired adds read q2[:, i+L] (one past the
                # segment) whose result only lands in the repaired spill cell
                q2 = cpool.tile([128, S + 1], f32, name=f"q2_{c}")
                nc.vector.tensor_copy(p1[:], pm1[:])
                nc.vector.tensor_scalar_add(q2[:, :S], pm2[:], fcb_t[c])

                chunk_list = _stripe_chunks(c)
                pieces = _chunk_pieces(chunk_list)
                coff = 0
                for ci, csz in enumerate(chunk_list):
                    fused = [p for p in pieces[ci] if p[0] < fuse_t]
                    rest = [p for p in pieces[ci] if p[0] >= fuse_t]
                    ot2 = opool2.tile([128, BIGCHUNK], f32, name="ot2")
                    if rest:
                        # adds for the short segments (paired where legal),
                        # then one tanh over their contiguous extent
                        ot = opool.tile([128, BIGCHUNK], f32, name="ot")
                        k = 0
                        while k < len(rest):
                            i, s0, s1, d0 = rest[k]
                            L = s1 - s0
                            pair = False
                            if k + 1 < len(rest) and s0 == i and L == S - i:
                                i2, t0, t1, e0 = rest[k + 1]
                                pair = (
                                    i2 == i + 1
                                    and t0 == i2
                                    and (t1 - t0) == (S - i2)
                                    and e0 == d0 + L
                                    and (k + 2 < len(rest) or csz < BIGCHUNK)
                                )
                            if pair:
                                nc.vector.tensor_tensor(
                                    _sub_ap(ot, d0, [[L, 2], [1, L]]),
                                    _sub_ap(q2, s0, [[1, 2], [1, L]]),
                                    _sub_ap(p1, i, [[1, 2], [0, L]]),
                                    op=mybir.AluOpType.add,
                                )
                                k += 2
                            else:
                                nc.vector.tensor_scalar_add(
                                    ot[:, d0 : d0 + L],
                                    q2[:, s0:s1],
                                    p1[:, i : i + 1],
                                )
                                k += 1
                        r0 = rest[0][3]
                        r1 = rest[-1][3] + (rest[-1][2] - rest[-1][1])
                        nc.scalar.activation(ot2[:, r0:r1], ot[:, r0:r1], Tanh)
                    for (i, s0, s1, d0) in fused:
                        nc.scalar.activation(
                            ot2[:, d0 : d0 + (s1 - s0)],
                            q2[:, s0:s1],
                            Tanh,
                            bias=p1[:, i : i + 1],
                        )
                    nc.sync.dma_start(
                        out_d[c, :, coff : coff + csz], ot2[:, :csz]
                    )
                    coff += csz
    nc.compile()
    return nc


def _get_nc():
    if "nc" not in _NC_CACHE:
        _NC_CACHE["nc"] = _build_nc()
    return _NC_CACHE["nc"]


def _make_in_maps(hidden_state, fc_w, fc_b):
    import ml_dtypes

    in_maps = []
    for k in range(8):
        b, h0 = k // 2, 384 * (k % 2)
        inp16 = np.empty((H, IC16), dtype=ml_dtypes.bfloat16)
        inp16[:, :S] = hidden_state[b].T.astype(ml_dtypes.bfloat16)
        inp16[:, W1C : W1C + 384] = fc_w[h0 : h0 + 384, :H].T.astype(
            ml_dtypes.bfloat16
        )
        inp16[:, W2C : W2C + 384] = fc_w[h0 : h0 + 384, H:].T.astype(
            ml_dtypes.bfloat16
        )
        aux = np.zeros((H, 2), dtype=np.float32)
        aux[: 128 * OC, 0] = fc_b[h0 : h0 + 384]
        in_maps.append(dict(inp16=inp16, aux=aux))
    return in_maps


def kernel(hidden_state, fc_w, fc_b, _trace=False, **_trace_kwargs):
    from concourse.bass_utils import run_bass_kernel_spmd

    hidden_state = np.asarray(hidden_state, dtype=np.float32)
    fc_w = np.asarray(fc_w, dtype=np.float32)
    fc_b = np.asarray(fc_b, dtype=np.float32)

    in_maps = _make_in_maps(hidden_state, fc_w, fc_b)
    nc = _get_nc()
    res = run_bass_kernel_spmd(
        nc, in_maps, core_ids=list(range(8)), trace=_trace, **_trace_kwargs
    )
    LAST["res"] = res

    full = np.empty((B, H, P), dtype=np.float32)
    for k in range(8):
        b, h0 = k // 2, 384 * (k % 2)
        full[b, h0 : h0 + 384] = res.results[k]["out"].reshape(384, P)
    return np.ascontiguousarray(full.transpose(0, 2, 1))


# revision 40
# speedup vs baseline: 2.7783x; 1.1229x over previous
"""Trainium2 Bass kernel for ConcatHandshaking.

out[b, p, :] = tanh(hidden[b, i_p] @ W1.T + hidden[b, j_p] @ W2.T + fc_b)
for the S*(S+1)/2 upper-triangular pairs (i_p, j_p), i-major order.

Device layout: output features (H=768) on SBUF partitions, pair index on the
free dim.  Then the pair-add is `q2T[:, j] + p1T[:, i]` where the second term
is a per-partition scalar -> one DVE tensor_scalar_add per triu segment,
fused bias, one big ACT tanh per output chunk, large contiguous DMA writes.

Sharding (8 cores): core k handles batch b = k//2 and output-feature rows
[384*(k%2), 384*(k%2)+384) -> 3 stripes of [128 features, 32896 pairs] each.
Per-core DRAM output is (3, 128, 32896); host reassembles + transposes.

Matmul operands ship as one bf16 tensor (PE 4x faster than f32; rel err
~1e-3 after f32 PSUM accumulation); fcb/zeros ship in a tiny f32 tensor.
The first stripe uses small leading chunks so the first output DMA starts
~12us in instead of waiting on a full 8224-wide chunk.
"""

import sys

import numpy as np

for _p in ("/opt/trn_rl_repo",):
    if _p not in sys.path:
        sys.path.insert(0, _p)

B, S, H = 4, 256, 768
P = S * (S + 1) // 2  # 32896
KT = H // 128  # 6 k-tiles
OC = 3  # o-chunks (of 128) per core
# bf16 packed matmul input columns: [ ht (S) | w1t (384) | w2t (384) ]
W1C = S
W2C = S + 128 * OC
IC16 = S + 2 * 128 * OC  # 1024
BIGCHUNK = 4112
SMALL = 2056
# segments with i < FUSE_T run as single ACT ops (tanh with per-partition
# bias = p1[:, i]) writing ot2 directly -- no DVE pass, no extra SBUF hops.
# Short segments (i >= FUSE_T) would drown in ACT instruction overhead, so
# they keep the add + one-big-tanh path on DVE.  (A GPSIMD band was tried
# and is ~6x slower per op on real HW than the cost model claims -- unused.)
# Consecutive full segments are merged in PAIRS into one DVE tensor_tensor
# with an overlapping-window AP: row g reads q2[i+g : i+g+L], adds
# p1[:, i+g] (free-step-0 broadcast).  Row 1 writes one spill element that
# the next instruction's first write repairs (same-engine program order).
FUSE_T = 32
GPS_LO = 128
GPS_HI = 128

_NC_CACHE = {}
LAST = {}


def _stripe_chunks(c):
    if c == 0:
        # small leading chunks: first output DMA launches early and the
        # stream never stalls waiting on one big chunk's DVE+ACT latency
        return [1028, 1028, 2056] + [BIGCHUNK] * 7
    return [BIGCHUNK] * 8


def _chunk_pieces(chunk_list):
    """Split triu segments along chunk boundaries.

    Returns per-chunk lists of (i, src0, src1, dst0):
    chunk[:, dst0:dst0+(src1-src0)] = q2T[:, src0:src1] + p1T[:, i].
    """
    bounds = [0]
    for sz in chunk_list:
        bounds.append(bounds[-1] + sz)
    assert bounds[-1] == P
    pieces = [[] for _ in chunk_list]
    off = 0
    for i in range(S):
        seg0, seg1 = off, off + (S - i)
        off = seg1
        for ci, (c0, c1) in enumerate(zip(bounds[:-1], bounds[1:])):
            s = max(seg0, c0)
            e = min(seg1, c1)
            if e > s:
                src0 = i + (s - seg0)  # free index in q2T is j itself
                pieces[ci].append((i, src0, src0 + (e - s), s - c0))
    return pieces


def _build_nc(loop_k=None, fuse_t=None, gps_lo=None, gps_hi=None):
    if fuse_t is None:
        fuse_t = FUSE_T
    if gps_lo is None:
        gps_lo = GPS_LO
    if gps_hi is None:
        gps_hi = GPS_HI
    import contextlib

    import concourse.bacc as bacc
    import concourse.bass as bass
    import concourse.mybir as mybir
    import concourse.tile as tile

    def _sub_ap(t, off, dims):
        return bass.AP(tensor=t.tensor, offset=t.offset + off, ap=[t.ap[0]] + dims)

    f32 = mybir.dt.float32
    bf16 = mybir.dt.bfloat16
    # Bacc (not raw Bass): its compile() runs generate_event_semaphores,
    # which splits multi-sem waits to satisfy TRN2's 1-wait-per-instruction.
    nc = bacc.Bacc()

    inp16_d = nc.declare_dram_parameter("inp16", [H, IC16], bf16, isOutput=False)
    # f32 side data: col 0 = fcb (rows 0:384), col 1 = zeros
    aux_d = nc.declare_dram_parameter("aux", [H, 2], f32, isOutput=False)
    out_d = nc.declare_dram_parameter("out", [OC, 128, P], f32, isOutput=True)

    Tanh = mybir.ActivationFunctionType.Tanh

    with tile.TileContext(nc) as tc:
        with (
            tc.tile_pool(name="const", bufs=1) as cpool,
            tc.tile_pool(name="mm", bufs=2, space="PSUM") as mpool,
            tc.tile_pool(name="outp", bufs=3) as opool,
            tc.tile_pool(name="outp2", bufs=6) as opool2,
            tc.For_i(0, loop_k, 1) if loop_k else contextlib.nullcontext(),
        ):
            # one DMA per k-tile so matmul kk can start as soon as its
            # k-tile lands (pipelines the load under the matmul chain)
            inp_b = cpool.tile([128, KT * IC16], bf16, name="inp_b")
            for kk in range(KT):
                nc.sync.dma_start(
                    inp_b[:, kk * IC16 : (kk + 1) * IC16],
                    inp16_d[kk * 128 : (kk + 1) * 128, :],
                )
            aux_b = cpool.tile([128, KT * 2], f32, name="aux_b")
            nc.sync.dma_start(
                aux_b[:].rearrange("p (t c) -> p t c", t=KT),
                aux_d.rearrange("(t p) c -> p t c", p=128),
            )
            # block kk occupies cols [kk*IC16, (kk+1)*IC16)
            ht_t = [inp_b[:, kk * IC16 : kk * IC16 + S] for kk in range(KT)]
            fcb_t = [aux_b[:, c * 2 : c * 2 + 1] for c in range(OC)]

            for c in range(OC):
                pm1 = mpool.tile([128, S], f32, name="pm1")
                pm2 = mpool.tile([128, S], f32, name="pm2")
                for kk in range(KT):
                    nc.tensor.matmul(
                        pm1[:],
                        inp_b[
                            :, kk * IC16 + W1C + c * 128 : kk * IC16 + W1C + (c + 1) * 128
                        ],
                        ht_t[kk],
                        start=(kk == 0),
                        stop=(kk == KT - 1),
                    )
                for kk in range(KT):
                    nc.tensor.matmul(
                        pm2[:],
                        inp_b[
                            :, kk * IC16 + W2C + c * 128 : kk * IC16 + W2C + (c + 1) * 128
                        ],
                        ht_t[kk],
                        start=(kk == 0),
                        stop=(kk == KT - 1),
                    )
                p1 = cpool.tile([128, S], f32, name=f"p1_{c}")
                # one pad column: paired adds read q2[:, i+L] (one past the
                # segment) whose result only lands in the repaired spill cell
                q2 = cpool.tile([128, S + 1], f32, name=f"q2_{c}")
                nc.vector.tensor_copy(p1[:], pm1[:])
                nc.vector.tensor_scalar_add(q2[:, :S], pm2[:], fcb_t[c])

                chunk_list = _stripe_chunks(c)
                pieces = _chunk_pieces(chunk_list)
                coff = 0
                for ci, csz in enumerate(chunk_list):
                    fused = [p for p in pieces[ci] if p[0] < fuse_t]
                    rest = [p for p in pieces[ci] if p[0] >= fuse_t]
                    ot2 = opool2.tile([128, BIGCHUNK], f32, name="ot2")
                    if rest:
                        # adds for the short segments (paired where legal),
                        # then one tanh over their contiguous extent
                        ot = opool.tile([128, BIGCHUNK], f32, name="ot")
                        k = 0
                        while k < len(rest):
                            i, s0, s1, d0 = rest[k]
                            L = s1 - s0
                            pair = False
                            if False and k + 1 < len(rest) and s0 == i and L == S - i:
                                i2, t0, t1, e0 = rest[k + 1]
                                pair = (
                                    i2 == i + 1
                                    and t0 == i2
                                    and (t1 - t0) == (S - i2)
                                    and e0 == d0 + L
                                    and (k + 2 < len(rest) or csz < BIGCHUNK)
                                )
                            if pair:
                                nc.vector.tensor_tensor(
                                    _sub_ap(ot, d0, [[L, 2], [1, L]]),
                                    _sub_ap(q2, s0, [[1, 2], [1, L]]),
                                    _sub_ap(p1, i, [[1, 2], [0, L]]),
                                    op=mybir.AluOpType.add,
                                )
                                k += 2
                            else:
                                nc.vector.tensor_scalar_add(
                                    ot[:, d0 : d0 + L],
                                    q2[:, s0:s1],
                                    p1[:, i : i + 1],
                                )
                                k += 1
                        r0 = rest[0][3]
                        r1 = rest[-1][3] + (rest[-1][2] - rest[-1][1])
                        nc.scalar.activation(ot2[:, r0:r1], ot[:, r0:r1], Tanh)
                    for (i, s0, s1, d0) in fused:
                        nc.scalar.activation(
                            ot2[:, d0 : d0 + (s1 - s0)],
                            q2[:, s0:s1],
                            Tanh,
                            bias=p1[:, i : i + 1],
                        )
                    nc.sync.dma_start(
                        out_d[c, :, coff : coff + csz], ot2[:, :csz]
                    )
                    coff += csz
    nc.compile()
    return nc


def _get_nc():
    if "nc" not in _NC_CACHE:
        _NC_CACHE["nc"] = _build_nc()
    return _NC_CACHE["nc"]


def _make_in_maps(hidden_state, fc_w, fc_b):
    import ml_dtypes

    in_maps = []
    for k in range(8):
        b, h0 = k // 2, 384 * (k % 2)
        inp16 = np.empty((H, IC16), dtype=ml_dtypes.bfloat16)
        inp16[:, :S] = hidden_state[b].T.astype(ml_dtypes.bfloat16)
        inp16[:, W1C : W1C + 384] = fc_w[h0 : h0 + 384, :H].T.astype(
            ml_dtypes.bfloat16
        )
        inp16[:, W2C : W2C + 384] = fc_w[h0 : h0 + 384, H:].T.astype(
            ml_dtypes.bfloat16
        )
        aux = np.zeros((H, 2), dtype=np.float32)
        aux[: 128 * OC, 0] = fc_b[h0 : h0 + 384]
        in_maps.append(dict(inp16=inp16, aux=aux))
    return in_maps


def kernel(hidden_state, fc_w, fc_b, _trace=False, **_trace_kwargs):
    from concourse.bass_utils import run_bass_kernel_spmd

    hidden_state = np.asarray(hidden_state, dtype=np.float32)
    fc_w = np.asarray(fc_w, dtype=np.float32)
    fc_b = np.asarray(fc_b, dtype=np.float32)

    in_maps = _make_in_maps(hidden_state, fc_w, fc_b)
    nc = _get_nc()
    res = run_bass_kernel_spmd(
        nc, in_maps, core_ids=list(range(8)), trace=_trace, **_trace_kwargs
    )
    LAST["res"] = res

    full = np.empty((B, H, P), dtype=np.float32)
    for k in range(8):
        b, h0 = k // 2, 384 * (k % 2)
        full[b, h0 : h0 + 384] = res.results[k]["out"].reshape(384, P)
    return np.ascontiguousarray(full.transpose(0, 2, 1))


# revision 41
# speedup vs baseline: 2.8198x; 1.0149x over previous
"""Trainium2 Bass kernel for ConcatHandshaking.

out[b, p, :] = tanh(hidden[b, i_p] @ W1.T + hidden[b, j_p] @ W2.T + fc_b)
for the S*(S+1)/2 upper-triangular pairs (i_p, j_p), i-major order.

Device layout: output features (H=768) on SBUF partitions, pair index on the
free dim.  Then the pair-add is `q2T[:, j] + p1T[:, i]` where the second term
is a per-partition scalar -> one DVE tensor_scalar_add per triu segment,
fused bias, one big ACT tanh per output chunk, large contiguous DMA writes.

Sharding (8 cores): core k handles batch b = k//2 and output-feature rows
[384*(k%2), 384*(k%2)+384) -> 3 stripes of [128 features, 32896 pairs] each.
Per-core DRAM output is (3, 128, 32896); host reassembles + transposes.

Matmul operands ship as one bf16 tensor (PE 4x faster than f32; rel err
~1e-3 after f32 PSUM accumulation); fcb/zeros ship in a tiny f32 tensor.
The first stripe uses small leading chunks so the first output DMA starts
~12us in instead of waiting on a full 8224-wide chunk.
"""

import sys

import numpy as np

for _p in ("/opt/trn_rl_repo",):
    if _p not in sys.path:
        sys.path.insert(0, _p)

B, S, H = 4, 256, 768
P = S * (S + 1) // 2  # 32896
KT = H // 128  # 6 k-tiles
OC = 3  # o-chunks (of 128) per core
# bf16 packed matmul input columns: [ ht (S) | w1t (384) | w2t (384) ]
W1C = S
W2C = S + 128 * OC
IC16 = S + 2 * 128 * OC  # 1024
BIGCHUNK = 2056
SMALL = 2056
# segments with i < FUSE_T run as single ACT ops (tanh with per-partition
# bias = p1[:, i]) writing ot2 directly -- no DVE pass, no extra SBUF hops.
# Short segments (i >= FUSE_T) would drown in ACT instruction overhead, so
# they keep the add + one-big-tanh path on DVE.  (A GPSIMD band was tried
# and is ~6x slower per op on real HW than the cost model claims -- unused.)
# Consecutive full segments are merged in PAIRS into one DVE tensor_tensor
# with an overlapping-window AP: row g reads q2[i+g : i+g+L], adds
# p1[:, i+g] (free-step-0 broadcast).  Row 1 writes one spill element that
# the next instruction's first write repairs (same-engine program order).
FUSE_T = 32
GPS_LO = 128
GPS_HI = 128

_NC_CACHE = {}
LAST = {}


def _stripe_chunks(c):
    if c == 0:
        # small leading chunks: first output DMA launches early and the
        # stream never stalls waiting on one big chunk's DVE+ACT latency
        return [1028, 1028] + [BIGCHUNK] * 15
    return [BIGCHUNK] * 16


def _chunk_pieces(chunk_list):
    """Split triu segments along chunk boundaries.

    Returns per-chunk lists of (i, src0, src1, dst0):
    chunk[:, dst0:dst0+(src1-src0)] = q2T[:, src0:src1] + p1T[:, i].
    """
    bounds = [0]
    for sz in chunk_list:
        bounds.append(bounds[-1] + sz)
    assert bounds[-1] == P
    pieces = [[] for _ in chunk_list]
    off = 0
    for i in range(S):
        seg0, seg1 = off, off + (S - i)
        off = seg1
        for ci, (c0, c1) in enumerate(zip(bounds[:-1], bounds[1:])):
            s = max(seg0, c0)
            e = min(seg1, c1)
            if e > s:
                src0 = i + (s - seg0)  # free index in q2T is j itself
                pieces[ci].append((i, src0, src0 + (e - s), s - c0))
    return pieces


def _build_nc(loop_k=None, fuse_t=None, gps_lo=None, gps_hi=None):
    if fuse_t is None:
        fuse_t = FUSE_T
    if gps_lo is None:
        gps_lo = GPS_LO
    if gps_hi is None:
        gps_hi = GPS_HI
    import contextlib

    import concourse.bacc as bacc
    import concourse.bass as bass
    import concourse.mybir as mybir
    import concourse.tile as tile

    def _sub_ap(t, off, dims):
        return bass.AP(tensor=t.tensor, offset=t.offset + off, ap=[t.ap[0]] + dims)

    f32 = mybir.dt.float32
    bf16 = mybir.dt.bfloat16
    # Bacc (not raw Bass): its compile() runs generate_event_semaphores,
    # which splits multi-sem waits to satisfy TRN2's 1-wait-per-instruction.
    nc = bacc.Bacc()

    inp16_d = nc.declare_dram_parameter("inp16", [H, IC16], bf16, isOutput=False)
    # f32 side data: col 0 = fcb (rows 0:384), col 1 = zeros
    aux_d = nc.declare_dram_parameter("aux", [H, 2], f32, isOutput=False)
    out_d = nc.declare_dram_parameter("out", [OC, 128, P], f32, isOutput=True)

    Tanh = mybir.ActivationFunctionType.Tanh

    with tile.TileContext(nc) as tc:
        with (
            tc.tile_pool(name="const", bufs=1) as cpool,
            tc.tile_pool(name="mm", bufs=2, space="PSUM") as mpool,
            tc.tile_pool(name="outp", bufs=6) as opool,
            tc.tile_pool(name="outp2", bufs=12) as opool2,
            tc.For_i(0, loop_k, 1) if loop_k else contextlib.nullcontext(),
        ):
            # one DMA per k-tile so matmul kk can start as soon as its
            # k-tile lands (pipelines the load under the matmul chain)
            inp_b = cpool.tile([128, KT * IC16], bf16, name="inp_b")
            for kk in range(KT):
                nc.sync.dma_start(
                    inp_b[:, kk * IC16 : (kk + 1) * IC16],
                    inp16_d[kk * 128 : (kk + 1) * 128, :],
                )
            aux_b = cpool.tile([128, KT * 2], f32, name="aux_b")
            nc.sync.dma_start(
                aux_b[:].rearrange("p (t c) -> p t c", t=KT),
                aux_d.rearrange("(t p) c -> p t c", p=128),
            )
            # block kk occupies cols [kk*IC16, (kk+1)*IC16)
            ht_t = [inp_b[:, kk * IC16 : kk * IC16 + S] for kk in range(KT)]
            fcb_t = [aux_b[:, c * 2 : c * 2 + 1] for c in range(OC)]

            for c in range(OC):
                pm1 = mpool.tile([128, S], f32, name="pm1")
                pm2 = mpool.tile([128, S], f32, name="pm2")
                for kk in range(KT):
                    nc.tensor.matmul(
                        pm1[:],
                        inp_b[
                            :, kk * IC16 + W1C + c * 128 : kk * IC16 + W1C + (c + 1) * 128
                        ],
                        ht_t[kk],
                        start=(kk == 0),
                        stop=(kk == KT - 1),
                    )
                for kk in range(KT):
                    nc.tensor.matmul(
                        pm2[:],
                        inp_b[
                            :, kk * IC16 + W2C + c * 128 : kk * IC16 + W2C + (c + 1) * 128
                        ],
                        ht_t[kk],
                        start=(kk == 0),
                        stop=(kk == KT - 1),
                    )
                p1 = cpool.tile([128, S], f32, name=f"p1_{c}")
                # one pad column: paired adds read q2[:, i+L] (one past the
                # segment) whose result only lands in the repaired spill cell
                q2 = cpool.tile([128, S + 1], f32, name=f"q2_{c}")
                nc.vector.tensor_copy(p1[:], pm1[:])
                nc.vector.tensor_scalar_add(q2[:, :S], pm2[:], fcb_t[c])

                chunk_list = _stripe_chunks(c)
                pieces = _chunk_pieces(chunk_list)
                coff = 0
                for ci, csz in enumerate(chunk_list):
                    fused = [p for p in pieces[ci] if p[0] < fuse_t]
                    rest = [p for p in pieces[ci] if p[0] >= fuse_t]
                    ot2 = opool2.tile([128, BIGCHUNK], f32, name="ot2")
                    if rest:
                        # adds for the short segments (paired where legal),
                        # then one tanh over their contiguous extent
                        ot = opool.tile([128, BIGCHUNK], f32, name="ot")
                        k = 0
                        while k < len(rest):
                            i, s0, s1, d0 = rest[k]
                            L = s1 - s0
                            pair = False
                            if False and k + 1 < len(rest) and s0 == i and L == S - i:
                                i2, t0, t1, e0 = rest[k + 1]
                                pair = (
                                    i2 == i + 1
                                    and t0 == i2
                                    and (t1 - t0) == (S - i2)
                                    and e0 == d0 + L
                                    and (k + 2 < len(rest) or csz < BIGCHUNK)
                                )
                            if pair:
                                nc.vector.tensor_tensor(
                                    _sub_ap(ot, d0, [[L, 2], [1, L]]),
                                    _sub_ap(q2, s0, [[1, 2], [1, L]]),
                                    _sub_ap(p1, i, [[1, 2], [0, L]]),
                                    op=mybir.AluOpType.add,
                                )
                                k += 2
                            else:
                                nc.vector.tensor_scalar_add(
                                    ot[:, d0 : d0 + L],
                                    q2[:, s0:s1],
                                    p1[:, i : i + 1],
                                )
                                k += 1
                        r0 = rest[0][3]
                        r1 = rest[-1][3] + (rest[-1][2] - rest[-1][1])
                        nc.scalar.activation(ot2[:, r0:r1], ot[:, r0:r1], Tanh)
                    for (i, s0, s1, d0) in fused:
                        nc.scalar.activation(
                            ot2[:, d0 : d0 + (s1 - s0)],
                            q2[:, s0:s1],
                            Tanh,
                            bias=p1[:, i : i + 1],
                        )
                    nc.sync.dma_start(
                        out_d[c, :, coff : coff + csz], ot2[:, :csz]
                    )
                    coff += csz
    nc.compile()
    return nc


def _get_nc():
    if "nc" not in _NC_CACHE:
        _NC_CACHE["nc"] = _build_nc()
    return _NC_CACHE["nc"]


def _make_in_maps(hidden_state, fc_w, fc_b):
    import ml_dtypes

    in_maps = []
    for k in range(8):
        b, h0 = k // 2, 384 * (k % 2)
        inp16 = np.empty((H, IC16), dtype=ml_dtypes.bfloat16)
        inp16[:, :S] = hidden_state[b].T.astype(ml_dtypes.bfloat16)
        inp16[:, W1C : W1C + 384] = fc_w[h0 : h0 + 384, :H].T.astype(
            ml_dtypes.bfloat16
        )
        inp16[:, W2C : W2C + 384] = fc_w[h0 : h0 + 384, H:].T.astype(
            ml_dtypes.bfloat16
        )
        aux = np.zeros((H, 2), dtype=np.float32)
        aux[: 128 * OC, 0] = fc_b[h0 : h0 + 384]
        in_maps.append(dict(inp16=inp16, aux=aux))
    return in_maps


def kernel(hidden_state, fc_w, fc_b, _trace=False, **_trace_kwargs):
    from concourse.bass_utils import run_bass_kernel_spmd

    hidden_state = np.asarray(hidden_state, dtype=np.float32)
    fc_w = np.asarray(fc_w, dtype=np.float32)
    fc_b = np.asarray(fc_b, dtype=np.float32)

    in_maps = _make_in_maps(hidden_state, fc_w, fc_b)
    nc = _get_nc()
    res = run_bass_kernel_spmd(
        nc, in_maps, core_ids=list(range(8)), trace=_trace, **_trace_kwargs
    )
    LAST["res"] = res

    full = np.empty((B, H, P), dtype=np.float32)
    for k in range(8):
        b, h0 = k // 2, 384 * (k % 2)
        full[b, h0 : h0 + 384] = res.results[k]["out"].reshape(384, P)
    return np.ascontiguousarray(full.transpose(0, 2, 1))
